# revision 15
# baseline (speedup 1.0000x reference)
"""Trainium2 Bass kernel for nn_Mlp_2_Layer (moe_routing).

Strategy: data-parallel over the batch. Each of the 8 NeuronCores takes
B/8 = 1024 samples and runs them through all D=8 per-domain MLPs.
Training-mode BatchNorm statistics span the full batch, so per-core
partial stats (mean, E[x^2]) are combined with small AllReduce
collectives, pipelined at domain-PAIR granularity so stats latency
hides under the next pair's matmuls.

Device pipeline per core (single L1 pass — h1 kept in SBUF as bf16):

  1. Embedding gather: TWO batched indirect DMAs (1024 rows each, one
     per 4-batch-tile group) from a flattened bf16 [F*V, E] table.
  2. XBAR DMA-transpose the gathered [batch, feature] tiles into
     xt [512 features, 1024 batch] bf16 (8 transpose DMAs, no PE).
  3. L1 per domain-pair: pre1 = W1 @ X (bf16 matmuls, f32 PSUM);
     bn_stats per [128,512] tile (vector); raw pre1 copied to SBUF
     h1[d] as bf16 (Pool engine nt=0 / Scalar nt=1). Per-pair
     (mean, E[x^2]) -> AllReduce.
  4. L2 per pair once its L1 stats return: a1 = relu(s1*h1+t1) fused
     on Scalar, h2pre = W2 @ a1, bn_stats again -> per-pair AllReduce;
     h2pre kept in SBUF bf16.
  5. Output per pair once its L2 stats return: a2 = relu(s2*h2+t2),
     dot with W3 via [128->1] matmuls accumulated over H2, raw logits
     staged to a [1, D*BC] row, DMA'd out.

Host combines: final[b] = sigmoid(z[domain_id[b], b] + b3[domain_id[b]]).

Emission order interleaves pairs (L1p0 L1p1 L2p0 L1p2 L2p1 P4p0 L1p3
L2p2 P4p1 L2p3 P4p2 P4p3) so the PE never waits on a collective.
"""
import sys

for _p in ("/opt/trn_rl_repo", "/root/.axon_site"):
    if _p not in sys.path:
        sys.path.insert(0, _p)

import numpy as np

B, F, E, V = 8192, 16, 32, 100000
D, H1, H2 = 8, 1024, 512
IN = F * E          # 512
EPS = 1e-5
NCORES = 8
BC = B // NCORES    # 1024 samples per core
NBT = BC // 128     # 8 batch tiles per core
P = 128
NT = BC // 512      # 2 n-chunks of 512 per core
K1 = IN // P        # 4 k-tiles for layer 1
M1 = H1 // P        # 8 m-tiles for layer 1
K2 = H1 // P        # 8 k-tiles for layer 2
M2 = H2 // P        # 4 m-tiles for layer 2
PD = 2              # domains per stats group (pair)
NP_ = D // PD       # 4 pairs

PROFILE = False       # test.py sets kernel.PROFILE = True
LAST_EXEC_NS = None   # filled when PROFILE

_NC = None


def _build():
    import concourse.bass as bass
    import concourse.tile as tile
    from concourse import bacc, mybir
    from contextlib import ExitStack

    f32 = mybir.dt.float32
    bf16 = mybir.dt.bfloat16
    i32 = mybir.dt.int32
    AF = mybir.ActivationFunctionType
    ALU = mybir.AluOpType

    nc = bacc.Bacc(None, target_bir_lowering=False, debug=False)

    tab_d = nc.dram_tensor("tab", [F * V, E], bf16, kind="ExternalInput")
    gidx_d = nc.dram_tensor("gidx", [P, NBT * F], i32, kind="ExternalInput")
    w1t_d = nc.dram_tensor("w1t", [D, IN, H1], bf16, kind="ExternalInput")
    w2t_d = nc.dram_tensor("w2t", [D, H1, H2], bf16, kind="ExternalInput")
    g1_d = nc.dram_tensor("g1", [D, H1], f32, kind="ExternalInput")
    be1_d = nc.dram_tensor("be1", [D, H1], f32, kind="ExternalInput")
    g2_d = nc.dram_tensor("g2", [D, H2], f32, kind="ExternalInput")
    be2_d = nc.dram_tensor("be2", [D, H2], f32, kind="ExternalInput")
    w3_d = nc.dram_tensor("w3", [D, H2], bf16, kind="ExternalInput")
    zout_d = nc.dram_tensor("zout", [1, D * BC], f32, kind="ExternalOutput")

    # collectives: 1 warm-up + 4 L1-pair + 4 L2-pair
    warm_in = nc.dram_tensor("ccwi", [1, 8], f32, kind="Internal")
    warm_out = nc.dram_tensor("ccwo", [1, 8], f32, kind="Internal",
                              addr_space="Shared")
    cc1_in = [nc.dram_tensor(f"cc1i{p}", [P, 2 * PD * M1], f32, kind="Internal")
              for p in range(NP_)]
    cc1_out = [nc.dram_tensor(f"cc1o{p}", [P, 2 * PD * M1], f32,
                              kind="Internal", addr_space="Shared")
               for p in range(NP_)]
    cc2_in = [nc.dram_tensor(f"cc2i{p}", [P, 2 * PD * M2], f32, kind="Internal")
              for p in range(NP_)]
    cc2_out = [nc.dram_tensor(f"cc2o{p}", [P, 2 * PD * M2], f32,
                              kind="Internal", addr_space="Shared")
               for p in range(NP_)]
    RG = [list(range(NCORES))]

    with tile.TileContext(nc) as tc:
        with ExitStack() as ctx:
            const = ctx.enter_context(tc.tile_pool(name="const", bufs=1))
            gp = ctx.enter_context(tc.tile_pool(name="gp", bufs=4))
            xtp = ctx.enter_context(tc.tile_pool(name="xtp", bufs=1))
            wp = ctx.enter_context(tc.tile_pool(name="wp", bufs=2))
            h1ap = ctx.enter_context(tc.tile_pool(name="h1ap", bufs=8))
            h1bp = ctx.enter_context(tc.tile_pool(name="h1bp", bufs=4))
            a1p = ctx.enter_context(tc.tile_pool(name="a1p", bufs=1))
            h2p = ctx.enter_context(tc.tile_pool(name="h2p", bufs=4))
            a2p = ctx.enter_context(tc.tile_pool(name="a2p", bufs=1))
            stp = ctx.enter_context(tc.tile_pool(name="stp", bufs=1))
            zsl = ctx.enter_context(tc.tile_pool(name="zsl", bufs=1))
            ps1 = ctx.enter_context(tc.tile_pool(name="ps1", bufs=3, space="PSUM"))
            ps2 = ctx.enter_context(tc.tile_pool(name="ps2", bufs=3, space="PSUM"))
            pso = ctx.enter_context(tc.tile_pool(name="pso", bufs=2, space="PSUM"))

            eps_t = const.tile([P, 1], f32, tag="eps")
            nc.vector.memset(eps_t[:], EPS)

            gidx = const.tile([P, NBT * F], i32, tag="gidx")
            nc.sync.dma_start(out=gidx[:], in_=gidx_d[:, :])

            g1c = const.tile([P, D * M1], f32, tag="g1c")
            nc.sync.dma_start(out=g1c[:], in_=g1_d[:, :].rearrange(
                "d (m p) -> p (d m)", p=P))
            be1c = const.tile([P, D * M1], f32, tag="be1c")
            nc.sync.dma_start(out=be1c[:], in_=be1_d[:, :].rearrange(
                "d (m p) -> p (d m)", p=P))
            g2c = const.tile([P, D * M2], f32, tag="g2c")
            nc.sync.dma_start(out=g2c[:], in_=g2_d[:, :].rearrange(
                "d (m p) -> p (d m)", p=P))
            be2c = const.tile([P, D * M2], f32, tag="be2c")
            nc.sync.dma_start(out=be2c[:], in_=be2_d[:, :].rearrange(
                "d (m p) -> p (d m)", p=P))
            w3r = const.tile([P, D * M2], bf16, tag="w3r")
            nc.sync.dma_start(out=w3r[:], in_=w3_d[:, :].rearrange(
                "d (m p) -> p (d m)", p=P))

            # warm up the collectives engine with a tiny AllReduce so the
            # first real stats reduce doesn't eat the ~30us cold start
            wtile = const.tile([1, 8], f32, tag="wtile")
            nc.vector.memset(wtile[:], 0.0)
            nc.gpsimd.dma_start(out=warm_in[:, :], in_=wtile[:])
            nc.gpsimd.collective_compute(
                "AllReduce", ALU.add, replica_groups=RG,
                ins=[warm_in[:, :]], outs=[warm_out[:, :]])

            # ---- Gather (per-(tile,feature) indirect DMAs; the batched
            # multi-column-offset form is semantically broken in the DGE
            # ucode). XBAR transposes ride the Scalar HWDGE queue; tiles
            # 0-3 transpose immediately, 4-7 after phase A (queue order).
            # xt is split per nt-chunk so L1 phase A starts as soon as the
            # first half of the batch is gathered. ----
            xts = [xtp.tile([P, K1, 512], bf16, tag=f"xt{i}", name=f"xt{i}")
                   for i in range(NT)]
            Gt = []
            for t in range(NBT):
                G = gp.tile([P, F, E], bf16, tag="G", name=f"G{t}")
                for f in range(F):
                    nc.gpsimd.indirect_dma_start(
                        out=G[:, f, :],
                        out_offset=None,
                        in_=tab_d[:, :],
                        in_offset=bass.IndirectOffsetOnAxis(
                            ap=gidx[:, t * F + f:t * F + f + 1], axis=0),
                    )
                Gt.append(G)

            def emit_xbar(t):
                nc.scalar.dma_start_transpose(
                    out=xts[t // 4][:, :, (t % 4) * P:(t % 4 + 1) * P],
                    in_=Gt[t][:].rearrange("p f e -> p (f e)"))

            for t in range(4):
                emit_xbar(t)

            # ---- per-pair stats tiles ----
            st1 = [stp.tile([P, PD, M1, NT, 6], f32, tag=f"st1_{p}", name=f"st1_{p}")
                   for p in range(NP_)]
            mv1 = [stp.tile([P, PD, M1, 2], f32, tag=f"mv1_{p}", name=f"mv1_{p}")
                   for p in range(NP_)]
            st2 = [stp.tile([P, PD, M2, NT, 6], f32, tag=f"st2_{p}", name=f"st2_{p}")
                   for p in range(NP_)]
            mv2 = [stp.tile([P, PD, M2, 2], f32, tag=f"mv2_{p}", name=f"mv2_{p}")
                   for p in range(NP_)]
            uq1 = [stp.tile([P, 2 * PD * M1], f32, tag=f"uq1_{p}", name=f"uq1_{p}")
                   for p in range(NP_)]
            sa1 = [stp.tile([P, 2 * PD * M1], f32, tag=f"sa1_{p}", name=f"sa1_{p}")
                   for p in range(NP_)]
            uq2 = [stp.tile([P, 2 * PD * M2], f32, tag=f"uq2_{p}", name=f"uq2_{p}")
                   for p in range(NP_)]
            sa2 = [stp.tile([P, 2 * PD * M2], f32, tag=f"sa2_{p}", name=f"sa2_{p}")
                   for p in range(NP_)]
            s1t = [stp.tile([P, PD * M1], f32, tag=f"s1_{p}", name=f"s1_{p}") for p in range(NP_)]
            t1t = [stp.tile([P, PD * M1], f32, tag=f"t1_{p}", name=f"t1_{p}") for p in range(NP_)]
            s2t = [stp.tile([P, PD * M2], f32, tag=f"s2_{p}", name=f"s2_{p}") for p in range(NP_)]
            t2t = [stp.tile([P, PD * M2], f32, tag=f"t2_{p}", name=f"t2_{p}") for p in range(NP_)]
            tmp1 = [stp.tile([P, PD * M1], f32, tag=f"tm_{p}", name=f"tm_{p}") for p in range(NP_)]
            tmp2 = [stp.tile([P, PD * M2], f32, tag=f"tn_{p}", name=f"tn_{p}") for p in range(NP_)]

            h1a = [None] * D
            h1b = [None] * D
            h2 = [None] * D
            w1 = [None] * D
            w2 = [None] * D

            def load_w1(d):
                w1[d] = wp.tile([P, K1, H1], bf16, tag="w1", name=f"w1_{d}")
                nc.sync.dma_start(
                    out=w1[d][:], in_=w1t_d[d, :, :].rearrange(
                        "(k p) h -> p k h", p=P))

            def load_w2(d):
                w2[d] = wp.tile([P, K2, H2], bf16, tag="w2", name=f"w2_{d}")
                nc.sync.dma_start(
                    out=w2[d][:], in_=w2t_d[d, :, :].rearrange(
                        "(k p) h -> p k h", p=P))

            def stats_pack(p, M, uq, mv, cci, cco):
                """Pack per-core (mean, E[x^2]) and kick the AllReduce."""
                n = PD * M
                u = uq[:, 0:n].rearrange("p (d m) -> p d m", d=PD)
                q = uq[:, n:].rearrange("p (d m) -> p d m", d=PD)
                nc.vector.tensor_copy(out=u, in_=mv[:, :, :, 0])
                nc.vector.tensor_mul(out=q, in0=mv[:, :, :, 0],
                                     in1=mv[:, :, :, 0])
                nc.vector.tensor_add(out=q, in0=q, in1=mv[:, :, :, 1])
                nc.gpsimd.dma_start(out=cci[:, :], in_=uq[:])
                nc.gpsimd.collective_compute(
                    "AllReduce", ALU.add, replica_groups=RG,
                    ins=[cci[:, :]], outs=[cco[:, :]])

            def stats_apply(p, M, sa, tmp, g_c, be_c, s_t, t_t):
                n = PD * M
                gl = slice(2 * p * M, (2 * p + PD) * M)
                mean = tmp[:, 0:n]
                var = sa[:, n:]
                nc.vector.tensor_scalar_mul(mean, sa[:, 0:n], 1.0 / NCORES)
                nc.vector.tensor_scalar_mul(var, var, 1.0 / NCORES)
                nc.vector.tensor_mul(out=s_t[:], in0=mean, in1=mean)
                nc.vector.tensor_tensor(out=var, in0=var, in1=s_t[:],
                                        op=ALU.subtract)
                nc.scalar.activation(out=var, in_=var, func=AF.Sqrt,
                                     bias=eps_t[:], scale=1.0)
                nc.vector.reciprocal(out=var, in_=var)
                nc.vector.tensor_mul(out=s_t[:], in0=g_c[:, gl], in1=var)
                nc.vector.tensor_mul(out=t_t[:], in0=mean, in1=s_t[:])
                nc.vector.tensor_tensor(out=t_t[:], in0=be_c[:, gl],
                                        in1=t_t[:], op=ALU.subtract)

            def emit_L1_nt(p, nt):
                """One nt-chunk of L1 for a pair. Phase A (nt=0) runs for
                all pairs while the second half of the batch gathers;
                phase B (nt=1) completes each pair's full-batch stats.
                w1 is re-streamed per phase (bufs=2 ring)."""
                for d in (PD * p, PD * p + 1):
                    load_w1(d)
                    if nt == 0:
                        h1a[d] = h1ap.tile([P, M1, 512], bf16, tag="h1a",
                                           name=f"h1a_{d}")
                    else:
                        h1b[d] = h1bp.tile([P, M1, 512], bf16, tag="h1b",
                                           name=f"h1b_{d}")
                for d in (PD * p, PD * p + 1):
                    dst = h1a[d] if nt == 0 else h1b[d]
                    for m in range(M1):
                        pm = ps1.tile([P, 512], f32, tag="ps1")
                        for k in range(K1):
                            nc.tensor.matmul(
                                out=pm[:],
                                lhsT=w1[d][:, k, m * P:(m + 1) * P],
                                rhs=xts[nt][:, k, :],
                                start=(k == 0), stop=(k == K1 - 1))
                        # PSUM -> SBUF bf16 (split across engines);
                        # bn_stats then reads the bf16 copy at 2x DVE rate
                        if nt == 0:
                            nc.scalar.activation(
                                out=dst[:, m, :], in_=pm[:], func=AF.Copy)
                        else:
                            nc.vector.tensor_copy(
                                out=dst[:, m, :], in_=pm[:])
                        nc.vector.bn_stats(
                            out=st1[p][:, d % PD, m, nt, :],
                            in_=dst[:, m, :])
                if nt == NT - 1:
                    for d in (PD * p, PD * p + 1):
                        for m in range(M1):
                            nc.vector.bn_aggr(
                                out=mv1[p][:, d % PD, m, :],
                                in_=st1[p][:, d % PD, m, :, :])
                    stats_pack(p, M1, uq1[p], mv1[p], cc1_in[p], cc1_out[p])

            def fetch_sa1(p):
                nc.sync.dma_start(out=sa1[p][:], in_=cc1_out[p][:, :])

            def fetch_sa2(p):
                nc.sync.dma_start(out=sa2[p][:], in_=cc2_out[p][:, :])

            def emit_L2(p):
                stats_apply(p, M1, sa1[p], tmp1[p], g1c, be1c, s1t[p], t1t[p])
                for d in (PD * p, PD * p + 1):
                    h2[d] = h2p.tile([P, M2, NT, 512], bf16, tag="h2", name=f"h2_{d}")
                    a1 = a1p.tile([P, K2, NT, 512], bf16, tag="a1")
                    for m in range(M1):
                        c = (d % PD) * M1 + m
                        nc.scalar.activation(
                            out=a1[:, m, 0, :], in_=h1a[d][:, m, :],
                            func=AF.Relu,
                            bias=t1t[p][:, c:c + 1], scale=s1t[p][:, c:c + 1])
                        nc.scalar.activation(
                            out=a1[:, m, 1, :], in_=h1b[d][:, m, :],
                            func=AF.Relu,
                            bias=t1t[p][:, c:c + 1], scale=s1t[p][:, c:c + 1])
                    for m2 in range(M2):
                        for nt in range(NT):
                            pm2 = ps2.tile([P, 512], f32, tag="ps2")
                            for k2 in range(K2):
                                nc.tensor.matmul(
                                    out=pm2[:],
                                    lhsT=w2[d][:, k2, m2 * P:(m2 + 1) * P],
                                    rhs=a1[:, k2, nt, :],
                                    start=(k2 == 0), stop=(k2 == K2 - 1))
                            if nt == 0:
                                nc.scalar.activation(
                                    out=h2[d][:, m2, nt, :], in_=pm2[:],
                                    func=AF.Copy)
                            else:
                                nc.vector.tensor_copy(
                                    out=h2[d][:, m2, nt, :], in_=pm2[:])
                            nc.vector.bn_stats(
                                out=st2[p][:, d % PD, m2, nt, :],
                                in_=h2[d][:, m2, nt, :])
                for d in (PD * p, PD * p + 1):
                    for m2 in range(M2):
                        nc.vector.bn_aggr(
                            out=mv2[p][:, d % PD, m2, :],
                            in_=st2[p][:, d % PD, m2, :, :])
                stats_pack(p, M2, uq2[p], mv2[p], cc2_in[p], cc2_out[p])

            def emit_P4(p):
                stats_apply(p, M2, sa2[p], tmp2[p], g2c, be2c, s2t[p], t2t[p])
                for d in (PD * p, PD * p + 1):
                    a2 = a2p.tile([P, M2, NT, 512], bf16, tag="a2")
                    for m2 in range(M2):
                        c = (d % PD) * M2 + m2
                        nc.scalar.activation(
                            out=a2[:, m2, :, :], in_=h2[d][:, m2, :, :],
                            func=AF.Relu,
                            bias=t2t[p][:, c:c + 1], scale=s2t[p][:, c:c + 1])
                    for nt in range(NT):
                        po = pso.tile([1, 512], f32, tag="po")
                        for m2 in range(M2):
                            nc.tensor.matmul(
                                out=po[:],
                                lhsT=w3r[:, d * M2 + m2:d * M2 + m2 + 1],
                                rhs=a2[:, m2, nt, :],
                                start=(m2 == 0), stop=(m2 == M2 - 1))
                        zs = zsl.tile([1, 512], f32, tag="zs")
                        nc.scalar.activation(out=zs[:], in_=po[:], func=AF.Copy)
                        nc.sync.dma_start(
                            out=zout_d[0:1,
                                       d * BC + nt * 512:d * BC + (nt + 1) * 512],
                            in_=zs[:])

            # ---- pipelined emission ----
            # phase A: nt0 of all pairs (runs while tiles 4-7 gather)
            for p in range(NP_):
                emit_L1_nt(p, 0)
            # transposes for the second half (scalar queue, after phase-A
            # copies so they don't block the PSUM drain)
            for t in range(4, NBT):
                emit_xbar(t)
            # phase B: nt1 per pair just-in-time before that pair's L2
            # (keeps the h1b ring shallow and overlaps AR latency)
            for p in range(NP_):
                emit_L1_nt(p, 1)
            load_w2(0)
            load_w2(1)
            fetch_sa1(0)
            emit_L2(0)
            load_w2(2)
            load_w2(3)
            fetch_sa1(1)
            emit_L2(1)
            fetch_sa2(0)
            emit_P4(0)
            load_w2(4)
            load_w2(5)
            fetch_sa1(2)
            emit_L2(2)
            fetch_sa2(1)
            emit_P4(1)
            load_w2(6)
            load_w2(7)
            fetch_sa1(3)
            emit_L2(3)
            fetch_sa2(2)
            emit_P4(2)
            fetch_sa2(3)
            emit_P4(3)

    nc.compile()
    return nc


def _prep_inputs(inputs):
    import ml_dtypes
    bf = ml_dtypes.bfloat16

    feat_ids = np.asarray(inputs["feat_ids"])
    emb_tables = np.asarray(inputs["emb_tables"], dtype=np.float32)
    W1 = np.asarray(inputs["W1"], dtype=np.float32)
    g1 = np.asarray(inputs["g1"], dtype=np.float32)
    be1 = np.asarray(inputs["be1"], dtype=np.float32)
    W2 = np.asarray(inputs["W2"], dtype=np.float32)
    g2 = np.asarray(inputs["g2"], dtype=np.float32)
    be2 = np.asarray(inputs["be2"], dtype=np.float32)
    W3 = np.asarray(inputs["W3"], dtype=np.float32)

    tab = np.ascontiguousarray(emb_tables.reshape(F * V, E).astype(bf))
    w1t = np.ascontiguousarray(W1.transpose(0, 2, 1).astype(bf))  # [D, IN, H1]
    w2t = np.ascontiguousarray(W2.transpose(0, 2, 1).astype(bf))  # [D, H1, H2]
    w3 = np.ascontiguousarray(W3.astype(bf))

    ids = feat_ids.astype(np.int64)
    in_maps = []
    for c in range(NCORES):
        idc = ids[c * BC:(c + 1) * BC]                   # [BC, F]
        g = idc.reshape(NBT, P, F).transpose(1, 0, 2)
        g = g + (np.arange(F, dtype=np.int64) * V)[None, None, :]
        gidx = np.ascontiguousarray(g.reshape(P, NBT * F).astype(np.int32))
        in_maps.append({
            "tab": tab, "gidx": gidx,
            "w1t": w1t, "w2t": w2t,
            "g1": g1, "be1": be1, "g2": g2, "be2": be2,
            "w3": w3,
        })
    return in_maps


def kernel(**inputs):
    global _NC, LAST_EXEC_NS
    from concourse.bass_utils import run_bass_kernel_spmd

    domain_id = np.asarray(inputs["domain_id"]).astype(np.int64)
    b3 = np.asarray(inputs["b3"], dtype=np.float32)

    if _NC is None:
        _NC = _build()

    in_maps = _prep_inputs(inputs)

    res = run_bass_kernel_spmd(
        _NC, in_maps, core_ids=list(range(NCORES)), trace=bool(PROFILE))
    if PROFILE:
        LAST_EXEC_NS = res.exec_time_ns
        globals()["LAST_INSTS"] = (
            res.instructions_and_trace[0]
            if res.instructions_and_trace is not None else None)

    z_full = np.concatenate(
        [res.results[c]["zout"].reshape(D, BC) for c in range(NCORES)],
        axis=1)                                          # [D, B]
    zsel = z_full[domain_id, np.arange(B)] + b3[domain_id]
    final = 1.0 / (1.0 + np.exp(-zsel))
    return final.astype(np.float32)


# revision 16
# speedup vs baseline: 1.1597x; 1.1597x over previous
"""Trainium2 Bass kernel for nn_Mlp_2_Layer (moe_routing).

Strategy: data-parallel over the batch. Each of the 8 NeuronCores takes
B/8 = 1024 samples and runs them through all D=8 per-domain MLPs.
Training-mode BatchNorm statistics span the full batch, so per-core
partial stats (mean, E[x^2]) are combined with small AllReduce
collectives, pipelined at domain-PAIR granularity so stats latency
hides under the next pair's matmuls.

Device pipeline per core (single L1 pass — h1 kept in SBUF as bf16):

  1. Embedding gather: TWO batched indirect DMAs (1024 rows each, one
     per 4-batch-tile group) from a flattened bf16 [F*V, E] table.
  2. XBAR DMA-transpose the gathered [batch, feature] tiles into
     xt [512 features, 1024 batch] bf16 (8 transpose DMAs, no PE).
  3. L1 per domain-pair: pre1 = W1 @ X (bf16 matmuls, f32 PSUM);
     bn_stats per [128,512] tile (vector); raw pre1 copied to SBUF
     h1[d] as bf16 (Pool engine nt=0 / Scalar nt=1). Per-pair
     (mean, E[x^2]) -> AllReduce.
  4. L2 per pair once its L1 stats return: a1 = relu(s1*h1+t1) fused
     on Scalar, h2pre = W2 @ a1, bn_stats again -> per-pair AllReduce;
     h2pre kept in SBUF bf16.
  5. Output per pair once its L2 stats return: a2 = relu(s2*h2+t2),
     dot with W3 via [128->1] matmuls accumulated over H2, raw logits
     staged to a [1, D*BC] row, DMA'd out.

Host combines: final[b] = sigmoid(z[domain_id[b], b] + b3[domain_id[b]]).

Emission order interleaves pairs (L1p0 L1p1 L2p0 L1p2 L2p1 P4p0 L1p3
L2p2 P4p1 L2p3 P4p2 P4p3) so the PE never waits on a collective.
"""
import sys

for _p in ("/opt/trn_rl_repo", "/root/.axon_site"):
    if _p not in sys.path:
        sys.path.insert(0, _p)

import numpy as np

B, F, E, V = 8192, 16, 32, 100000
D, H1, H2 = 8, 1024, 512
IN = F * E          # 512
EPS = 1e-5
NCORES = 8
BC = B // NCORES    # 1024 samples per core
NBT = BC // 128     # 8 batch tiles per core
P = 128
NT = BC // 512      # 2 n-chunks of 512 per core
K1 = IN // P        # 4 k-tiles for layer 1
M1 = H1 // P        # 8 m-tiles for layer 1
K2 = H1 // P        # 8 k-tiles for layer 2
M2 = H2 // P        # 4 m-tiles for layer 2
PD = 2              # domains per stats group (pair)
NP_ = D // PD       # 4 pairs

PROFILE = False       # test.py sets kernel.PROFILE = True
LAST_EXEC_NS = None   # filled when PROFILE

_NC = None


def _build():
    import concourse.bass as bass
    import concourse.tile as tile
    from concourse import bacc, mybir
    from contextlib import ExitStack

    f32 = mybir.dt.float32
    bf16 = mybir.dt.bfloat16
    i32 = mybir.dt.int32
    AF = mybir.ActivationFunctionType
    ALU = mybir.AluOpType

    nc = bacc.Bacc(None, target_bir_lowering=False, debug=False)

    tab_d = nc.dram_tensor("tab", [F * V, E], bf16, kind="ExternalInput")
    gidx_d = nc.dram_tensor("gidx", [P, NBT * F], i32, kind="ExternalInput")
    w1t_d = nc.dram_tensor("w1t", [D, IN, H1], bf16, kind="ExternalInput")
    w2t_d = nc.dram_tensor("w2t", [D, H1, H2], bf16, kind="ExternalInput")
    g1_d = nc.dram_tensor("g1", [D, H1], f32, kind="ExternalInput")
    be1_d = nc.dram_tensor("be1", [D, H1], f32, kind="ExternalInput")
    g2_d = nc.dram_tensor("g2", [D, H2], f32, kind="ExternalInput")
    be2_d = nc.dram_tensor("be2", [D, H2], f32, kind="ExternalInput")
    w3_d = nc.dram_tensor("w3", [D, H2], bf16, kind="ExternalInput")
    zout_d = nc.dram_tensor("zout", [1, D * BC], f32, kind="ExternalOutput")

    # collectives: 1 warm-up + 4 L1-pair + 4 L2-pair
    warm_in = nc.dram_tensor("ccwi", [1, 8], f32, kind="Internal")
    warm_out = nc.dram_tensor("ccwo", [1, 8], f32, kind="Internal",
                              addr_space="Shared")
    cc1_in = [nc.dram_tensor(f"cc1i{p}", [P, 2 * PD * M1], f32, kind="Internal")
              for p in range(NP_)]
    cc1_out = [nc.dram_tensor(f"cc1o{p}", [P, 2 * PD * M1], f32,
                              kind="Internal", addr_space="Shared")
               for p in range(NP_)]
    cc2_in = [nc.dram_tensor(f"cc2i{p}", [P, 2 * PD * M2], f32, kind="Internal")
              for p in range(NP_)]
    cc2_out = [nc.dram_tensor(f"cc2o{p}", [P, 2 * PD * M2], f32,
                              kind="Internal", addr_space="Shared")
               for p in range(NP_)]
    RG = [list(range(NCORES))]

    with tile.TileContext(nc) as tc:
        with ExitStack() as ctx:
            const = ctx.enter_context(tc.tile_pool(name="const", bufs=1))
            gp = ctx.enter_context(tc.tile_pool(name="gp", bufs=4))
            xtp = ctx.enter_context(tc.tile_pool(name="xtp", bufs=1))
            wp = ctx.enter_context(tc.tile_pool(name="wp", bufs=2))
            h1ap = ctx.enter_context(tc.tile_pool(name="h1ap", bufs=8))
            h1bp = ctx.enter_context(tc.tile_pool(name="h1bp", bufs=4))
            a1p = ctx.enter_context(tc.tile_pool(name="a1p", bufs=1))
            h2p = ctx.enter_context(tc.tile_pool(name="h2p", bufs=4))
            a2p = ctx.enter_context(tc.tile_pool(name="a2p", bufs=1))
            stp = ctx.enter_context(tc.tile_pool(name="stp", bufs=1))
            zsl = ctx.enter_context(tc.tile_pool(name="zsl", bufs=1))
            ps1 = ctx.enter_context(tc.tile_pool(name="ps1", bufs=3, space="PSUM"))
            ps2 = ctx.enter_context(tc.tile_pool(name="ps2", bufs=3, space="PSUM"))
            pso = ctx.enter_context(tc.tile_pool(name="pso", bufs=2, space="PSUM"))

            eps_t = const.tile([P, 1], f32, tag="eps")
            nc.vector.memset(eps_t[:], EPS)

            gidx = const.tile([P, NBT * F], i32, tag="gidx")
            nc.sync.dma_start(out=gidx[:], in_=gidx_d[:, :])

            g1c = const.tile([P, D * M1], f32, tag="g1c")
            nc.sync.dma_start(out=g1c[:], in_=g1_d[:, :].rearrange(
                "d (m p) -> p (d m)", p=P))
            be1c = const.tile([P, D * M1], f32, tag="be1c")
            nc.sync.dma_start(out=be1c[:], in_=be1_d[:, :].rearrange(
                "d (m p) -> p (d m)", p=P))
            g2c = const.tile([P, D * M2], f32, tag="g2c")
            nc.sync.dma_start(out=g2c[:], in_=g2_d[:, :].rearrange(
                "d (m p) -> p (d m)", p=P))
            be2c = const.tile([P, D * M2], f32, tag="be2c")
            nc.sync.dma_start(out=be2c[:], in_=be2_d[:, :].rearrange(
                "d (m p) -> p (d m)", p=P))
            w3r = const.tile([P, D * M2], bf16, tag="w3r")
            nc.sync.dma_start(out=w3r[:], in_=w3_d[:, :].rearrange(
                "d (m p) -> p (d m)", p=P))

            # warm up the collectives engine with a tiny AllReduce so the
            # first real stats reduce doesn't eat the ~30us cold start
            wtile = const.tile([1, 8], f32, tag="wtile")
            nc.vector.memset(wtile[:], 0.0)
            nc.gpsimd.dma_start(out=warm_in[:, :], in_=wtile[:])
            nc.gpsimd.collective_compute(
                "AllReduce", ALU.add, replica_groups=RG,
                ins=[warm_in[:, :]], outs=[warm_out[:, :]])

            # ---- Gather (per-(tile,feature) indirect DMAs; the batched
            # multi-column-offset form is semantically broken in the DGE
            # ucode). XBAR transposes ride the Scalar HWDGE queue; tiles
            # 0-3 transpose immediately, 4-7 after phase A (queue order).
            # xt is split per nt-chunk so L1 phase A starts as soon as the
            # first half of the batch is gathered. ----
            xts = [xtp.tile([P, K1, 512], bf16, tag=f"xt{i}", name=f"xt{i}")
                   for i in range(NT)]
            Gt = []
            for t in range(NBT):
                G = gp.tile([P, F, E], bf16, tag="G", name=f"G{t}")
                for f in range(F):
                    nc.gpsimd.indirect_dma_start(
                        out=G[:, f, :],
                        out_offset=None,
                        in_=tab_d[:, :],
                        in_offset=bass.IndirectOffsetOnAxis(
                            ap=gidx[:, t * F + f:t * F + f + 1], axis=0),
                    )
                Gt.append(G)

            def emit_xbar(t):
                nc.scalar.dma_start_transpose(
                    out=xts[t // 4][:, :, (t % 4) * P:(t % 4 + 1) * P],
                    in_=Gt[t][:].rearrange("p f e -> p (f e)"))

            for t in range(4):
                emit_xbar(t)

            # ---- per-pair stats tiles ----
            st1 = [stp.tile([P, PD, M1, NT, 6], f32, tag=f"st1_{p}", name=f"st1_{p}")
                   for p in range(NP_)]
            mv1 = [stp.tile([P, PD, M1, 2], f32, tag=f"mv1_{p}", name=f"mv1_{p}")
                   for p in range(NP_)]
            st2 = [stp.tile([P, PD, M2, NT, 6], f32, tag=f"st2_{p}", name=f"st2_{p}")
                   for p in range(NP_)]
            mv2 = [stp.tile([P, PD, M2, 2], f32, tag=f"mv2_{p}", name=f"mv2_{p}")
                   for p in range(NP_)]
            uq1 = [stp.tile([P, 2 * PD * M1], f32, tag=f"uq1_{p}", name=f"uq1_{p}")
                   for p in range(NP_)]
            sa1 = [stp.tile([P, 2 * PD * M1], f32, tag=f"sa1_{p}", name=f"sa1_{p}")
                   for p in range(NP_)]
            uq2 = [stp.tile([P, 2 * PD * M2], f32, tag=f"uq2_{p}", name=f"uq2_{p}")
                   for p in range(NP_)]
            sa2 = [stp.tile([P, 2 * PD * M2], f32, tag=f"sa2_{p}", name=f"sa2_{p}")
                   for p in range(NP_)]
            s1t = [stp.tile([P, PD * M1], f32, tag=f"s1_{p}", name=f"s1_{p}") for p in range(NP_)]
            t1t = [stp.tile([P, PD * M1], f32, tag=f"t1_{p}", name=f"t1_{p}") for p in range(NP_)]
            s2t = [stp.tile([P, PD * M2], f32, tag=f"s2_{p}", name=f"s2_{p}") for p in range(NP_)]
            t2t = [stp.tile([P, PD * M2], f32, tag=f"t2_{p}", name=f"t2_{p}") for p in range(NP_)]
            tmp1 = [stp.tile([P, PD * M1], f32, tag=f"tm_{p}", name=f"tm_{p}") for p in range(NP_)]
            tmp2 = [stp.tile([P, PD * M2], f32, tag=f"tn_{p}", name=f"tn_{p}") for p in range(NP_)]

            h1a = [None] * D
            h1b = [None] * D
            h2 = [None] * D
            w1 = [None] * D
            w2 = [None] * D

            def load_w1(d):
                w1[d] = wp.tile([P, K1, H1], bf16, tag="w1", name=f"w1_{d}")
                nc.sync.dma_start(
                    out=w1[d][:], in_=w1t_d[d, :, :].rearrange(
                        "(k p) h -> p k h", p=P))

            def load_w2(d):
                w2[d] = wp.tile([P, K2, H2], bf16, tag="w2", name=f"w2_{d}")
                nc.sync.dma_start(
                    out=w2[d][:], in_=w2t_d[d, :, :].rearrange(
                        "(k p) h -> p k h", p=P))

            def stats_pack(p, M, uq, mv, cci, cco):
                """Pack per-core (mean, E[x^2]) and kick the AllReduce."""
                n = PD * M
                u = uq[:, 0:n].rearrange("p (d m) -> p d m", d=PD)
                q = uq[:, n:].rearrange("p (d m) -> p d m", d=PD)
                nc.vector.tensor_copy(out=u, in_=mv[:, :, :, 0])
                nc.vector.tensor_mul(out=q, in0=mv[:, :, :, 0],
                                     in1=mv[:, :, :, 0])
                nc.vector.tensor_add(out=q, in0=q, in1=mv[:, :, :, 1])
                nc.gpsimd.dma_start(out=cci[:, :], in_=uq[:])
                nc.gpsimd.collective_compute(
                    "AllReduce", ALU.add, replica_groups=RG,
                    ins=[cci[:, :]], outs=[cco[:, :]])

            def stats_apply(p, M, sa, tmp, g_c, be_c, s_t, t_t):
                n = PD * M
                gl = slice(2 * p * M, (2 * p + PD) * M)
                mean = tmp[:, 0:n]
                var = sa[:, n:]
                nc.vector.tensor_scalar_mul(mean, sa[:, 0:n], 1.0 / NCORES)
                nc.vector.tensor_scalar_mul(var, var, 1.0 / NCORES)
                nc.vector.tensor_mul(out=s_t[:], in0=mean, in1=mean)
                nc.vector.tensor_tensor(out=var, in0=var, in1=s_t[:],
                                        op=ALU.subtract)
                nc.scalar.activation(out=var, in_=var, func=AF.Sqrt,
                                     bias=eps_t[:], scale=1.0)
                nc.vector.reciprocal(out=var, in_=var)
                nc.vector.tensor_mul(out=s_t[:], in0=g_c[:, gl], in1=var)
                nc.vector.tensor_mul(out=t_t[:], in0=mean, in1=s_t[:])
                nc.vector.tensor_tensor(out=t_t[:], in0=be_c[:, gl],
                                        in1=t_t[:], op=ALU.subtract)

            def emit_L1_nt(p, nt):
                """One nt-chunk of L1 for a pair. Phase A (nt=0) runs for
                all pairs while the second half of the batch gathers;
                phase B (nt=1) completes each pair's full-batch stats.
                w1 is re-streamed per phase (bufs=2 ring)."""
                for d in (PD * p, PD * p + 1):
                    load_w1(d)
                    if nt == 0:
                        h1a[d] = h1ap.tile([P, M1, 512], bf16, tag="h1a",
                                           name=f"h1a_{d}")
                    else:
                        h1b[d] = h1bp.tile([P, M1, 512], bf16, tag="h1b",
                                           name=f"h1b_{d}")
                for d in (PD * p, PD * p + 1):
                    dst = h1a[d] if nt == 0 else h1b[d]
                    for m in range(M1):
                        pm = ps1.tile([P, 512], f32, tag="ps1")
                        for k in range(K1):
                            nc.tensor.matmul(
                                out=pm[:],
                                lhsT=w1[d][:, k, m * P:(m + 1) * P],
                                rhs=xts[nt][:, k, :],
                                start=(k == 0), stop=(k == K1 - 1))
                        # PSUM -> SBUF bf16 (split across engines);
                        # bn_stats then reads the bf16 copy at 2x DVE rate
                        if nt == 0:
                            nc.scalar.activation(
                                out=dst[:, m, :], in_=pm[:], func=AF.Copy)
                        else:
                            nc.vector.tensor_copy(
                                out=dst[:, m, :], in_=pm[:])
                        nc.vector.bn_stats(
                            out=st1[p][:, d % PD, m, nt, :],
                            in_=dst[:, m, :])
                if nt == NT - 1:
                    for d in (PD * p, PD * p + 1):
                        for m in range(M1):
                            nc.vector.bn_aggr(
                                out=mv1[p][:, d % PD, m, :],
                                in_=st1[p][:, d % PD, m, :, :])
                    stats_pack(p, M1, uq1[p], mv1[p], cc1_in[p], cc1_out[p])

            def fetch_sa1(p):
                nc.sync.dma_start(out=sa1[p][:], in_=cc1_out[p][:, :])

            def fetch_sa2(p):
                nc.sync.dma_start(out=sa2[p][:], in_=cc2_out[p][:, :])

            def emit_L2(p):
                stats_apply(p, M1, sa1[p], tmp1[p], g1c, be1c, s1t[p], t1t[p])
                for d in (PD * p, PD * p + 1):
                    h2[d] = h2p.tile([P, M2, NT, 512], bf16, tag="h2", name=f"h2_{d}")
                    # per-nt a1 tiles + nt-outer matmuls: the next domain's
                    # nt0 activations overlap this domain's nt1 matmuls
                    a1x = a1p.tile([P, K2, 512], bf16, tag="a1x", name=f"a1x_{d}")
                    a1y = a1p.tile([P, K2, 512], bf16, tag="a1y", name=f"a1y_{d}")
                    for m in range(M1):
                        c = (d % PD) * M1 + m
                        nc.scalar.activation(
                            out=a1x[:, m, :], in_=h1a[d][:, m, :],
                            func=AF.Relu,
                            bias=t1t[p][:, c:c + 1], scale=s1t[p][:, c:c + 1])
                        nc.scalar.activation(
                            out=a1y[:, m, :], in_=h1b[d][:, m, :],
                            func=AF.Relu,
                            bias=t1t[p][:, c:c + 1], scale=s1t[p][:, c:c + 1])
                    for nt in range(NT):
                        a1c = a1x if nt == 0 else a1y
                        for m2 in range(M2):
                            pm2 = ps2.tile([P, 512], f32, tag="ps2")
                            for k2 in range(K2):
                                nc.tensor.matmul(
                                    out=pm2[:],
                                    lhsT=w2[d][:, k2, m2 * P:(m2 + 1) * P],
                                    rhs=a1c[:, k2, :],
                                    start=(k2 == 0), stop=(k2 == K2 - 1))
                            if nt == 0:
                                nc.scalar.activation(
                                    out=h2[d][:, m2, nt, :], in_=pm2[:],
                                    func=AF.Copy)
                            else:
                                nc.vector.tensor_copy(
                                    out=h2[d][:, m2, nt, :], in_=pm2[:])
                            nc.vector.bn_stats(
                                out=st2[p][:, d % PD, m2, nt, :],
                                in_=h2[d][:, m2, nt, :])
                for d in (PD * p, PD * p + 1):
                    for m2 in range(M2):
                        nc.vector.bn_aggr(
                            out=mv2[p][:, d % PD, m2, :],
                            in_=st2[p][:, d % PD, m2, :, :])
                stats_pack(p, M2, uq2[p], mv2[p], cc2_in[p], cc2_out[p])

            def emit_P4(p):
                stats_apply(p, M2, sa2[p], tmp2[p], g2c, be2c, s2t[p], t2t[p])
                for d in (PD * p, PD * p + 1):
                    a2 = a2p.tile([P, M2, NT, 512], bf16, tag="a2")
                    for m2 in range(M2):
                        c = (d % PD) * M2 + m2
                        nc.scalar.activation(
                            out=a2[:, m2, :, :], in_=h2[d][:, m2, :, :],
                            func=AF.Relu,
                            bias=t2t[p][:, c:c + 1], scale=s2t[p][:, c:c + 1])
                    for nt in range(NT):
                        po = pso.tile([1, 512], f32, tag="po")
                        for m2 in range(M2):
                            nc.tensor.matmul(
                                out=po[:],
                                lhsT=w3r[:, d * M2 + m2:d * M2 + m2 + 1],
                                rhs=a2[:, m2, nt, :],
                                start=(m2 == 0), stop=(m2 == M2 - 1))
                        zs = zsl.tile([1, 512], f32, tag="zs")
                        nc.scalar.activation(out=zs[:], in_=po[:], func=AF.Copy)
                        nc.sync.dma_start(
                            out=zout_d[0:1,
                                       d * BC + nt * 512:d * BC + (nt + 1) * 512],
                            in_=zs[:])

            # ---- pipelined emission ----
            # phase A: nt0 of all pairs (runs while tiles 4-7 gather)
            for p in range(NP_):
                emit_L1_nt(p, 0)
            # transposes for the second half (scalar queue, after phase-A
            # copies so they don't block the PSUM drain)
            for t in range(4, NBT):
                emit_xbar(t)
            # phase B: nt1 per pair just-in-time before that pair's L2
            # (keeps the h1b ring shallow and overlaps AR latency)
            for p in range(NP_):
                emit_L1_nt(p, 1)
            load_w2(0)
            load_w2(1)
            fetch_sa1(0)
            emit_L2(0)
            load_w2(2)
            load_w2(3)
            fetch_sa1(1)
            emit_L2(1)
            fetch_sa2(0)
            emit_P4(0)
            load_w2(4)
            load_w2(5)
            fetch_sa1(2)
            emit_L2(2)
            fetch_sa2(1)
            emit_P4(1)
            load_w2(6)
            load_w2(7)
            fetch_sa1(3)
            emit_L2(3)
            fetch_sa2(2)
            emit_P4(2)
            fetch_sa2(3)
            emit_P4(3)

    nc.compile()
    return nc


def _prep_inputs(inputs):
    import ml_dtypes
    bf = ml_dtypes.bfloat16

    feat_ids = np.asarray(inputs["feat_ids"])
    emb_tables = np.asarray(inputs["emb_tables"], dtype=np.float32)
    W1 = np.asarray(inputs["W1"], dtype=np.float32)
    g1 = np.asarray(inputs["g1"], dtype=np.float32)
    be1 = np.asarray(inputs["be1"], dtype=np.float32)
    W2 = np.asarray(inputs["W2"], dtype=np.float32)
    g2 = np.asarray(inputs["g2"], dtype=np.float32)
    be2 = np.asarray(inputs["be2"], dtype=np.float32)
    W3 = np.asarray(inputs["W3"], dtype=np.float32)

    tab = np.ascontiguousarray(emb_tables.reshape(F * V, E).astype(bf))
    w1t = np.ascontiguousarray(W1.transpose(0, 2, 1).astype(bf))  # [D, IN, H1]
    w2t = np.ascontiguousarray(W2.transpose(0, 2, 1).astype(bf))  # [D, H1, H2]
    w3 = np.ascontiguousarray(W3.astype(bf))

    ids = feat_ids.astype(np.int64)
    in_maps = []
    for c in range(NCORES):
        idc = ids[c * BC:(c + 1) * BC]                   # [BC, F]
        g = idc.reshape(NBT, P, F).transpose(1, 0, 2)
        g = g + (np.arange(F, dtype=np.int64) * V)[None, None, :]
        gidx = np.ascontiguousarray(g.reshape(P, NBT * F).astype(np.int32))
        in_maps.append({
            "tab": tab, "gidx": gidx,
            "w1t": w1t, "w2t": w2t,
            "g1": g1, "be1": be1, "g2": g2, "be2": be2,
            "w3": w3,
        })
    return in_maps


def kernel(**inputs):
    global _NC, LAST_EXEC_NS
    from concourse.bass_utils import run_bass_kernel_spmd

    domain_id = np.asarray(inputs["domain_id"]).astype(np.int64)
    b3 = np.asarray(inputs["b3"], dtype=np.float32)

    if _NC is None:
        _NC = _build()

    in_maps = _prep_inputs(inputs)

    res = run_bass_kernel_spmd(
        _NC, in_maps, core_ids=list(range(NCORES)), trace=bool(PROFILE))
    if PROFILE:
        LAST_EXEC_NS = res.exec_time_ns
        globals()["LAST_INSTS"] = (
            res.instructions_and_trace[0]
            if res.instructions_and_trace is not None else None)

    z_full = np.concatenate(
        [res.results[c]["zout"].reshape(D, BC) for c in range(NCORES)],
        axis=1)                                          # [D, B]
    zsel = z_full[domain_id, np.arange(B)] + b3[domain_id]
    final = 1.0 / (1.0 + np.exp(-zsel))
    return final.astype(np.float32)
